# revision 7
# baseline (speedup 1.0000x reference)
"""Trainium2 Bass kernel for nn_CluserAssignment (vq_codebook).

Computes, for X (N, D) and a one-hot cluster_mask (N, C):
    counts  = sum_n mask[n, c]
    mu      = (mask.T @ X) / counts                     (C, D)
    dist    = sum_d |X[n, d] - mu[c, d]|                (N, C)  L1
    q       = 1 / (1 + dist)
    norm_q  = q / sum_c q                               (N, C)

Strategy (8 NeuronCores, data parallel over N):
  - Each core takes N_loc = N/8 = 2048 rows of X and mask.
  - Phase A: local  sums^T = X^T @ mask  (PE matmuls, K=n on partitions)
    and replicated counts  = ones^T @ mask; AllReduce(sums^T | counts);
    negmu^T = -sums^T / counts  (d on partitions, c on free dim).
  - X^T on-chip via PE transpose (fp32 DMA transpose unsupported).
  - Phase B: for each cluster c: |x - mu_c| computed with d on partitions
    in ONE pass per engine:
       ACT:  Abs(x * 1 + (-mu_c))        (per-partition bias)
       DVE:  tensor_scalar((x + (-mu_c)) abs_max 0)  (two per-part scalars)
    then the sum over d (partitions) via TensorE: absdiff (K=128, M=128)
    as lhsT against a ones column, accumulating (128 pts, 1) into a PSUM
    tile laid out (128, NT, C) -> distance block (points, clusters).
  - Epilogue: q = 1/(1+dist), row-normalize over the free (cluster) dim,
    DMA out.
"""

import os
from contextlib import ExitStack

import numpy as np

import concourse.bacc as bacc
import concourse.bass as bass
import concourse.mybir as mybir
import concourse.tile as tile
from concourse.bass_utils import run_bass_kernel_spmd
from concourse.masks import make_identity

N, C, D = 16384, 64, 256
NCORES = 8
NLOC = N // NCORES  # 2048
P = 128
NT = NLOC // P  # 16 row-tiles per core
KD = D // P  # 2 d-chunks

f32 = mybir.dt.float32

# absdiff dtype fed to the PE reduction. fp16 halves PE weight-load time
# (2-byte weights load 2 rows/cycle) at ~1e-5 relative output error.
ABS_DT = mybir.dt.float16

# Of every 8 clusters, this many go to the DVE producer; rest to ACT.
# DVE fp32 tensor_scalar (2x_2p) ~ 2 elem/cy/lane @0.96GHz vs ACT 1 @1.2GHz.
DVE_OF_8 = 5


def _build_nc() -> bass.Bass:
    nc = bacc.Bacc(
        "TRN2", target_bir_lowering=False, debug=False, num_devices=NCORES
    )

    X = nc.declare_dram_parameter("X", [NLOC, D], f32, isOutput=False)
    M = nc.declare_dram_parameter("cluster_mask", [NLOC, C], f32, isOutput=False)
    OUT = nc.declare_dram_parameter("out", [NLOC, C], f32, isOutput=True)

    with tile.TileContext(nc) as tc, ExitStack() as ctx:
        singles = ctx.enter_context(tc.tile_pool(name="singles", bufs=1))
        psum = ctx.enter_context(tc.tile_pool(name="psum", bufs=1, space="PSUM"))
        ad_pool = ctx.enter_context(tc.tile_pool(name="ad", bufs=4))
        ep = ctx.enter_context(tc.tile_pool(name="ep", bufs=1))
        dram = ctx.enter_context(tc.tile_pool(name="dram", bufs=1, space="DRAM"))

        # ---- loads -------------------------------------------------------
        x_sb = singles.tile([P, NT, D], f32)
        msk_sb = singles.tile([P, NT, C], f32)
        nc.sync.dma_start(out=x_sb, in_=X.rearrange("(t p) d -> p t d", p=P))
        nc.sync.dma_start(out=msk_sb, in_=M.rearrange("(t p) c -> p t c", p=P))

        onesmat = singles.tile([P, P], f32)
        nc.vector.memset(onesmat, 1.0)
        ident = singles.tile([P, P], f32)
        make_identity(nc, ident)
        two_mov = singles.tile([P, 1], ABS_DT)
        nc.vector.memset(two_mov, 2.0)
        ones_f32col = singles.tile([P, 1], f32)
        nc.vector.memset(ones_f32col, 1.0)

        # ---- phase A: local cluster sums + counts ------------------------
        # Separate PSUM tiles (each padded to its own bank) so the three
        # accumulation chains can interleave without zero-region conflicts.
        sums_ps = [
            psum.tile([P, C], f32, tag=f"sums{k}", name=f"sums{k}")
            for k in range(KD)
        ]
        cnt_ps = psum.tile([P, C], f32, tag="cnt")  # counts on all partitions
        for t in range(NT):
            for k in range(KD):
                nc.tensor.matmul(
                    sums_ps[k],
                    lhsT=x_sb[:, t, P * k : P * (k + 1)],
                    rhs=msk_sb[:, t, :],
                    start=(t == 0),
                    stop=(t == NT - 1),
                )
            nc.tensor.matmul(
                cnt_ps,
                lhsT=onesmat,
                rhs=msk_sb[:, t, :],
                start=(t == 0),
                stop=(t == NT - 1),
            )

        stage = singles.tile([P, (KD + 1) * C], f32)
        for k in range(KD):
            nc.scalar.copy(stage[:, k * C : (k + 1) * C], sums_ps[k])
        nc.scalar.copy(stage[:, KD * C :], cnt_ps)

        red_in = dram.tile([P, (KD + 1) * C], f32)
        red_out = dram.tile([P, (KD + 1) * C], f32)
        nc.sync.dma_start(out=red_in, in_=stage)
        nc.gpsimd.collective_compute(
            "AllReduce",
            mybir.AluOpType.add,
            replica_groups=[list(range(NCORES))],
            ins=[red_in.opt()],
            outs=[red_out.opt()],
        )
        gsb = singles.tile([P, (KD + 1) * C], f32)
        nc.sync.dma_start(out=gsb, in_=red_out)

        # negmuT[:, k, c] = -sumsT[d, c] / counts[c]
        negrec = singles.tile([P, C], f32)
        nc.scalar.mul(negrec, gsb[:, KD * C :], -1.0)
        nc.vector.reciprocal(negrec, negrec)
        negmuT = singles.tile([P, KD, C], f32)
        for k in range(KD):
            nc.vector.tensor_mul(
                negmuT[:, k, :], gsb[:, k * C : (k + 1) * C], negrec
            )

        # ---- X^T via PE transpose ---------------------------------------
        # tp_ps spans 4 PSUM banks; 4 transposes land per bank, forming one
        # ordered chain per bank: `start` only on the bank's first write (it
        # lazily zeroes the whole 2KB zero region), `stop` on its last.
        xt_sb = singles.tile([P, KD, NLOC], f32)
        TPB = 4  # (128,128) f32 transposes per PSUM bank
        for k in range(KD):
            tp_ps = psum.tile([P, NLOC], f32, tag="big")
            for t in range(NT):
                nc.tensor.matmul(
                    tp_ps[:, P * t : P * (t + 1)],
                    lhsT=x_sb[:, t, P * k : P * (k + 1)],
                    rhs=ident,
                    is_transpose=True,
                    start=(t % TPB == 0),
                    stop=(t % TPB == TPB - 1),
                )
            nc.scalar.copy(xt_sb[:, k, :], tp_ps)

        # ---- correction terms for |v| = 2*relu(v) - v -------------------
        # dist = sum_d |x-mu_c| = 2*sum_d relu(x-mu_c) - Sx + Smu_c
        # Sx[n] = sum_d x[n,d]  -> psum_sx (128, NT), one chain in one bank
        psum_sx = psum.tile([P, NT], f32, tag="sums0")
        for t in range(NT):
            for k in range(KD):
                nc.tensor.matmul(
                    psum_sx[:, t : t + 1],
                    lhsT=xt_sb[:, k, P * t : P * (t + 1)],
                    rhs=ones_f32col,
                    start=(t == 0 and k == 0),
                    stop=(t == NT - 1 and k == KD - 1),
                )
        # s = 1 - Sx  (the 1 folds the "+1" of q = 1/(1+dist))
        s_sb = singles.tile([P, NT], f32)
        nc.scalar.activation(
            out=s_sb,
            in_=psum_sx,
            func=mybir.ActivationFunctionType.Copy,
            bias=1.0,
            scale=-1.0,
        )
        # -Smu_c broadcast over partitions: ones(128,128)^T @ negmuT
        psum_smu = psum.tile([P, C], f32, tag="sums1")
        for k in range(KD):
            nc.tensor.matmul(
                psum_smu,
                lhsT=onesmat,
                rhs=negmuT[:, k, :],
                start=(k == 0),
                stop=(k == KD - 1),
            )
        negsmu_sb = singles.tile([P, C], f32)
        nc.scalar.copy(negsmu_sb, psum_smu)

        # ---- phase B: 2 * sum_d relu(x - mu_c) --------------------------
        # dist_ps spans 2 PSUM banks (cols t*C+c; bank = t//8). Each bank is
        # one long accumulation chain in program order: only its very first
        # matmul has start=True, only its last has stop=True. Per-byte
        # pending-zero then makes each column's first write an overwrite and
        # its second (other k chunk) an accumulate. The moving column is 2.0
        # so PSUM directly accumulates 2*R.
        dist_ps = psum.tile([P, NT, C], f32, tag="big")
        BANK_T = 8  # t-values per PSUM bank
        for c in range(C):
            ads = []
            use_dve = (c % 8) < DVE_OF_8
            for k in range(KD):
                ad = ad_pool.tile([P, NLOC], ABS_DT, tag="ad")
                scal = negmuT[:, k, c : c + 1]
                if use_dve:
                    nc.vector.tensor_scalar(
                        out=ad,
                        in0=xt_sb[:, k, :],
                        scalar1=scal,
                        scalar2=0.0,
                        op0=mybir.AluOpType.add,
                        op1=mybir.AluOpType.max,
                    )
                else:
                    nc.scalar.activation(
                        out=ad,
                        in_=xt_sb[:, k, :],
                        func=mybir.ActivationFunctionType.Relu,
                        bias=scal,
                        scale=1.0,
                    )
                ads.append(ad)
            for t in range(NT):
                for k in range(KD):
                    first = c == 0 and k == 0 and t % BANK_T == 0
                    last = (
                        c == C - 1
                        and k == KD - 1
                        and t % BANK_T == BANK_T - 1
                    )
                    nc.tensor.matmul(
                        dist_ps[:, t, c : c + 1],
                        lhsT=ads[k][:, P * t : P * (t + 1)],
                        rhs=two_mov,
                        start=first,
                        stop=last,
                    )

        # ---- epilogue: q = 1/(1+dist); normalize rows; store ------------
        # 1 + dist = (2R + (1 - Sx)) - (-Smu)
        q_sb = ep.tile([P, NT, C], f32)
        for t in range(NT):
            nc.vector.scalar_tensor_tensor(
                out=q_sb[:, t, :],
                in0=dist_ps[:, t, :],
                scalar=s_sb[:, t : t + 1],
                in1=negsmu_sb,
                op0=mybir.AluOpType.add,
                op1=mybir.AluOpType.subtract,
            )
        nc.vector.reciprocal(q_sb, q_sb)
        ssum = ep.tile([P, NT], f32)
        nc.vector.tensor_reduce(
            ssum, q_sb, axis=mybir.AxisListType.X, op=mybir.AluOpType.add
        )
        nc.vector.reciprocal(ssum, ssum)
        out_sb = ep.tile([P, NT, C], f32)
        for t in range(NT):
            nc.vector.tensor_scalar_mul(
                out_sb[:, t, :], q_sb[:, t, :], ssum[:, t : t + 1]
            )
        nc.sync.dma_start(
            out=OUT.rearrange("(t p) c -> p t c", p=P), in_=out_sb
        )

    nc.compile()
    return nc


_NC_CACHE: bass.Bass | None = None


def _get_nc() -> bass.Bass:
    global _NC_CACHE
    if _NC_CACHE is None:
        _NC_CACHE = _build_nc()
    return _NC_CACHE


def kernel(X: np.ndarray, cluster_mask: np.ndarray) -> np.ndarray:
    X = np.ascontiguousarray(X, dtype=np.float32)
    cluster_mask = np.ascontiguousarray(cluster_mask, dtype=np.float32)
    nc = _get_nc()
    in_maps = [
        {
            "X": X[c * NLOC : (c + 1) * NLOC],
            "cluster_mask": cluster_mask[c * NLOC : (c + 1) * NLOC],
        }
        for c in range(NCORES)
    ]
    res = run_bass_kernel_spmd(nc, in_maps, list(range(NCORES)))
    return np.concatenate([r["out"] for r in res.results], axis=0)


if __name__ == "__main__":
    rng = np.random.default_rng(0)
    X = rng.standard_normal((N, D), dtype=np.float32)
    ids = rng.permutation(np.arange(N) % C)
    mask = np.eye(C, dtype=np.float32)[ids]
    out = kernel(X, mask)
    print(out.shape, out.dtype, out.sum())


# revision 11
# speedup vs baseline: 7.5120x; 7.5120x over previous
"""Trainium2 Bass kernel for nn_CluserAssignment (vq_codebook).

Computes, for X (N, D) and a one-hot cluster_mask (N, C):
    counts  = sum_n mask[n, c]
    mu      = (mask.T @ X) / counts                     (C, D)
    dist    = sum_d |X[n, d] - mu[c, d]|                (N, C)  L1
    q       = 1 / (1 + dist)
    norm_q  = q / sum_c q                               (N, C)

Strategy (8 NeuronCores, data parallel over N):
  - Each core takes N_loc = N/8 = 2048 rows of X and mask.
  - Phase A: local  sums^T = X^T @ mask  (PE matmuls, K=n on partitions)
    and replicated counts  = ones^T @ mask; AllReduce(sums^T | counts);
    negmu^T = -sums^T / counts  (d on partitions, c on free dim).
  - X^T on-chip via PE transpose (fp32 DMA transpose unsupported).
  - Phase B: for each cluster c: |x - mu_c| computed with d on partitions
    in ONE pass per engine:
       ACT:  Abs(x * 1 + (-mu_c))        (per-partition bias)
       DVE:  tensor_scalar((x + (-mu_c)) abs_max 0)  (two per-part scalars)
    then the sum over d (partitions) via TensorE: absdiff (K=128, M=128)
    as lhsT against a ones column, accumulating (128 pts, 1) into a PSUM
    tile laid out (128, NT, C) -> distance block (points, clusters).
  - Epilogue: q = 1/(1+dist), row-normalize over the free (cluster) dim,
    DMA out.
"""

import os
from contextlib import ExitStack

import numpy as np

import concourse.bacc as bacc
import concourse.bass as bass
import concourse.mybir as mybir
import concourse.tile as tile
from concourse.bass_utils import run_bass_kernel_spmd
from concourse.masks import make_identity

N, C, D = 16384, 64, 256
NCORES = 8
NLOC = N // NCORES  # 2048
P = 128
NT = NLOC // P  # 16 row-tiles per core
KD = D // P  # 2 d-chunks

f32 = mybir.dt.float32

# absdiff dtype fed to the PE reduction. fp16 halves PE weight-load time
# (2-byte weights load 2 rows/cycle) at ~1e-5 relative output error.
ABS_DT = mybir.dt.float16

# When True (set by tsim.py before building), replace the AllReduce with a
# local DMA copy so the single-core cost-model timeline can run.
FAKE_COLLECTIVE = False

# Of every 8 clusters, this many go to the DVE producer; rest to ACT.
# DVE fp32 tensor_scalar (2x_2p) ~ 2 elem/cy/lane @0.96GHz vs ACT 1 @1.2GHz.
DVE_OF_8 = 5


def _build_nc(n_reps: int = 1) -> bass.Bass:
    nc = bacc.Bacc(
        "TRN2", target_bir_lowering=False, debug=False, num_devices=NCORES
    )

    X = nc.declare_dram_parameter("X", [NLOC, D], f32, isOutput=False)
    M = nc.declare_dram_parameter("cluster_mask", [NLOC, C], f32, isOutput=False)
    OUT = nc.declare_dram_parameter("out", [NLOC, C], f32, isOutput=True)

    with tile.TileContext(nc) as tc, ExitStack() as ctx:
        singles = ctx.enter_context(tc.tile_pool(name="singles", bufs=1))
        psum = ctx.enter_context(tc.tile_pool(name="psum", bufs=1, space="PSUM"))
        ad_pool = ctx.enter_context(tc.tile_pool(name="ad", bufs=4))
        ep = ctx.enter_context(tc.tile_pool(name="ep", bufs=1))
        dram = ctx.enter_context(tc.tile_pool(name="dram", bufs=1, space="DRAM"))

        consts = ctx.enter_context(tc.tile_pool(name="consts", bufs=1))
        onesmat = consts.tile([P, P], f32)
        nc.vector.memset(onesmat, 1.0)
        ident = consts.tile([P, P], f32)
        make_identity(nc, ident)
        two_mov = consts.tile([P, 1], ABS_DT)
        nc.vector.memset(two_mov, 2.0)
        ones_f32col = consts.tile([P, 1], f32)
        nc.vector.memset(ones_f32col, 1.0)

        for _rep in range(n_reps):
            _build_rep(
                nc, tc, singles, psum, ad_pool, ep, dram,
                X, M, OUT, onesmat, ident, two_mov, ones_f32col,
            )

    nc.compile()
    return nc


def _build_rep(
    nc, tc, singles, psum, ad_pool, ep, dram,
    X, M, OUT, onesmat, ident, two_mov, ones_f32col,
):
    if True:  # keep indentation of the original body
        # ---- loads -------------------------------------------------------
        x_sb = singles.tile([P, NT, D], f32, tag="x_sb", name="x_sb")
        msk_sb = singles.tile([P, NT, C], f32, tag="msk_sb", name="msk_sb")
        nc.sync.dma_start(out=x_sb, in_=X.rearrange("(t p) d -> p t d", p=P))
        nc.sync.dma_start(out=msk_sb, in_=M.rearrange("(t p) c -> p t c", p=P))

        # ---- phase A: local cluster sums + counts ------------------------
        # Separate PSUM tiles (each padded to its own bank) so the three
        # accumulation chains can interleave without zero-region conflicts.
        sums_ps = [
            psum.tile([P, C], f32, tag=f"sums{k}", name=f"sums{k}")
            for k in range(KD)
        ]
        cnt_ps = psum.tile([P, C], f32, tag="cnt")  # counts on all partitions
        for t in range(NT):
            for k in range(KD):
                nc.tensor.matmul(
                    sums_ps[k],
                    lhsT=x_sb[:, t, P * k : P * (k + 1)],
                    rhs=msk_sb[:, t, :],
                    start=(t == 0),
                    stop=(t == NT - 1),
                )
            nc.tensor.matmul(
                cnt_ps,
                lhsT=onesmat,
                rhs=msk_sb[:, t, :],
                start=(t == 0),
                stop=(t == NT - 1),
            )

        stage = singles.tile([P, (KD + 1) * C], f32)
        for k in range(KD):
            nc.scalar.copy(stage[:, k * C : (k + 1) * C], sums_ps[k])
        nc.scalar.copy(stage[:, KD * C :], cnt_ps)

        red_in = dram.tile([P, (KD + 1) * C], f32)
        red_out = dram.tile([P, (KD + 1) * C], f32)
        nc.sync.dma_start(out=red_in, in_=stage)
        if FAKE_COLLECTIVE:
            nc.sync.dma_start(out=red_out, in_=red_in)
        else:
            nc.gpsimd.collective_compute(
                "AllReduce",
                mybir.AluOpType.add,
                replica_groups=[list(range(NCORES))],
                ins=[red_in.opt()],
                outs=[red_out.opt()],
            )
        gsb = singles.tile([P, (KD + 1) * C], f32)
        nc.sync.dma_start(out=gsb, in_=red_out)

        # negmuT[:, k, c] = -sumsT[d, c] / counts[c]
        negrec = singles.tile([P, C], f32)
        nc.scalar.mul(negrec, gsb[:, KD * C :], -1.0)
        nc.vector.reciprocal(negrec, negrec)
        negmuT = singles.tile([P, KD, C], f32)
        for k in range(KD):
            nc.vector.tensor_mul(
                negmuT[:, k, :], gsb[:, k * C : (k + 1) * C], negrec
            )

        # ---- X^T via PE transpose ---------------------------------------
        # tp_ps spans 4 PSUM banks; 4 transposes land per bank, forming one
        # ordered chain per bank: `start` only on the bank's first write (it
        # lazily zeroes the whole 2KB zero region), `stop` on its last.
        xt_sb = singles.tile([P, KD, NLOC], f32)
        TPB = 4  # (128,128) f32 transposes per PSUM bank
        for k in range(KD):
            tp_ps = psum.tile([P, NLOC], f32, tag="big")
            for t in range(NT):
                nc.tensor.matmul(
                    tp_ps[:, P * t : P * (t + 1)],
                    lhsT=x_sb[:, t, P * k : P * (k + 1)],
                    rhs=ident,
                    is_transpose=True,
                    start=(t % TPB == 0),
                    stop=(t % TPB == TPB - 1),
                )
            nc.scalar.copy(xt_sb[:, k, :], tp_ps)

        # ---- correction terms for |v| = 2*relu(v) - v -------------------
        # dist = sum_d |x-mu_c| = 2*sum_d relu(x-mu_c) - Sx + Smu_c
        # Sx[n] = sum_d x[n,d]  -> psum_sx (128, NT), one chain in one bank
        psum_sx = psum.tile([P, NT], f32, tag="sums0")
        for t in range(NT):
            for k in range(KD):
                nc.tensor.matmul(
                    psum_sx[:, t : t + 1],
                    lhsT=xt_sb[:, k, P * t : P * (t + 1)],
                    rhs=ones_f32col,
                    start=(t == 0 and k == 0),
                    stop=(t == NT - 1 and k == KD - 1),
                )
        # s = 1 - Sx  (the 1 folds the "+1" of q = 1/(1+dist))
        s_sb = singles.tile([P, NT], f32)
        nc.scalar.activation(
            out=s_sb,
            in_=psum_sx,
            func=mybir.ActivationFunctionType.Copy,
            bias=1.0,
            scale=-1.0,
        )
        # -Smu_c broadcast over partitions: ones(128,128)^T @ negmuT
        psum_smu = psum.tile([P, C], f32, tag="sums1")
        for k in range(KD):
            nc.tensor.matmul(
                psum_smu,
                lhsT=onesmat,
                rhs=negmuT[:, k, :],
                start=(k == 0),
                stop=(k == KD - 1),
            )
        negsmu_sb = singles.tile([P, C], f32)
        nc.scalar.copy(negsmu_sb, psum_smu)

        # ---- phase B: 2 * sum_d relu(x - mu_c) --------------------------
        # dist_ps spans 2 PSUM banks (cols t*C+c; bank = t//8). Each bank is
        # one long accumulation chain in program order: only its very first
        # matmul has start=True, only its last has stop=True. Per-byte
        # pending-zero then makes each column's first write an overwrite and
        # its second (other k chunk) an accumulate. The moving column is 2.0
        # so PSUM directly accumulates 2*R.
        dist_ps = psum.tile([P, NT, C], f32, tag="big")
        BANK_T = 8  # t-values per PSUM bank
        for c in range(C):
            ads = []
            use_dve = (c % 8) < DVE_OF_8
            for k in range(KD):
                ad = ad_pool.tile([P, NLOC], ABS_DT, tag="ad")
                scal = negmuT[:, k, c : c + 1]
                if use_dve:
                    nc.vector.tensor_scalar(
                        out=ad,
                        in0=xt_sb[:, k, :],
                        scalar1=scal,
                        scalar2=0.0,
                        op0=mybir.AluOpType.add,
                        op1=mybir.AluOpType.max,
                    )
                else:
                    nc.scalar.activation(
                        out=ad,
                        in_=xt_sb[:, k, :],
                        func=mybir.ActivationFunctionType.Relu,
                        bias=scal,
                        scale=1.0,
                    )
                ads.append(ad)
            for t in range(NT):
                for k in range(KD):
                    first = c == 0 and k == 0 and t % BANK_T == 0
                    last = (
                        c == C - 1
                        and k == KD - 1
                        and t % BANK_T == BANK_T - 1
                    )
                    nc.tensor.matmul(
                        dist_ps[:, t, c : c + 1],
                        lhsT=ads[k][:, P * t : P * (t + 1)],
                        rhs=two_mov,
                        start=first,
                        stop=last,
                    )

        # ---- epilogue: q = 1/(1+dist); normalize rows; store ------------
        # 1 + dist = (2R + (1 - Sx)) - (-Smu)
        q_sb = ep.tile([P, NT, C], f32)
        for t in range(NT):
            nc.vector.scalar_tensor_tensor(
                out=q_sb[:, t, :],
                in0=dist_ps[:, t, :],
                scalar=s_sb[:, t : t + 1],
                in1=negsmu_sb,
                op0=mybir.AluOpType.add,
                op1=mybir.AluOpType.subtract,
            )
        nc.vector.reciprocal(q_sb, q_sb)
        ssum = ep.tile([P, NT], f32)
        nc.vector.tensor_reduce(
            ssum, q_sb, axis=mybir.AxisListType.X, op=mybir.AluOpType.add
        )
        nc.vector.reciprocal(ssum, ssum)
        out_sb = ep.tile([P, NT, C], f32)
        for t in range(NT):
            nc.vector.tensor_scalar_mul(
                out_sb[:, t, :], q_sb[:, t, :], ssum[:, t : t + 1]
            )
        nc.sync.dma_start(
            out=OUT.rearrange("(t p) c -> p t c", p=P), in_=out_sb
        )


_NC_CACHE: dict[int, bass.Bass] = {}


def _get_nc(n_reps: int = 1) -> bass.Bass:
    if n_reps not in _NC_CACHE:
        _NC_CACHE[n_reps] = _build_nc(n_reps)
    return _NC_CACHE[n_reps]


def kernel(X: np.ndarray, cluster_mask: np.ndarray) -> np.ndarray:
    X = np.ascontiguousarray(X, dtype=np.float32)
    cluster_mask = np.ascontiguousarray(cluster_mask, dtype=np.float32)
    nc = _get_nc()
    in_maps = [
        {
            "X": X[c * NLOC : (c + 1) * NLOC],
            "cluster_mask": cluster_mask[c * NLOC : (c + 1) * NLOC],
        }
        for c in range(NCORES)
    ]
    res = run_bass_kernel_spmd(nc, in_maps, list(range(NCORES)))
    return np.concatenate([r["out"] for r in res.results], axis=0)


if __name__ == "__main__":
    rng = np.random.default_rng(0)
    X = rng.standard_normal((N, D), dtype=np.float32)
    ids = rng.permutation(np.arange(N) % C)
    mask = np.eye(C, dtype=np.float32)[ids]
    out = kernel(X, mask)
    print(out.shape, out.dtype, out.sum())
